# revision 3
# baseline (speedup 1.0000x reference)
"""MoE (63 routed experts, top-7, 1 shared expert) Trainium2 Bass kernel.

Strategy (expert parallelism, per sharding hint):
  - Host: router matmul + softmax + top-k (tiny: 0.7 GFLOP vs 220 GFLOP of
    expert FFNs), token gather per expert.
  - Device (8 NeuronCores, SPMD): each core runs 9 "units" of identical
    shape: 8 routed-expert slots (64 slots globally = 63 experts + 1
    overflow slot) and 1 shared-expert slot over a 1/8 token slice.
    Each unit: h = gelu(XeT^T @ W1 + b1); y = gate * (h @ W2), with
    float32r full-rate matmuls, GELU fused into the PSUM eviction on the
    scalar engine, gating fused into the PSUM eviction on the vector engine.
  - Host: scatter-add gated expert outputs (+ gate*b2), add shared out,
    bias and residual.

Capacity C=512 tokens/slot (seed-0 max expert load is 520; the single
largest-overflow expert spills into the overflow slot; anything beyond that
falls back to an exact host-side FFN for the few excess tokens).
"""

import numpy as np

B, S, HID = 2, 2048, 1280
E = 63
I = 1280
TOP_K = 7
NCORES = 8
UNITS = 9          # 8 expert slots + 1 shared-expert slot
C = 512            # token capacity per expert slot
CM = C // 128      # 4
KO = HID // 128    # 10 contraction chunks
T = B * S          # 4096
TSH = T // NCORES  # 512 shared-expert tokens per core

_cache = {}


def _build_nc():
    import concourse.bass as bass
    import concourse.mybir as mybir
    import concourse.tile as tile
    from concourse import bacc

    f32 = mybir.dt.float32
    f32r = mybir.dt.float32r
    GELU = mybir.ActivationFunctionType.Gelu

    nc = bacc.Bacc(None, target_bir_lowering=False)

    xg_d = nc.dram_tensor("xg", [UNITS, 128, KO, C], f32, kind="ExternalInput")
    gates_d = nc.dram_tensor("gates", [UNITS, 128, CM], f32, kind="ExternalInput")
    w1_d = nc.dram_tensor("w1", [UNITS, HID, I], f32, kind="ExternalInput")
    b1_d = nc.dram_tensor("b1", [UNITS, 128, KO], f32, kind="ExternalInput")
    w2_d = nc.dram_tensor("w2", [UNITS, I, HID], f32, kind="ExternalInput")
    out_d = nc.dram_tensor("out", [UNITS, 128, CM, HID], f32, kind="ExternalOutput")

    W1CW = 256          # w1 chunk width along I (2 psum-column groups)
    W2CW = 256          # w2 chunk width along H
    N_W1C = I // W1CW   # 5
    N_W2C = HID // W2CW  # 5

    with tile.TileContext(nc) as tc:
        with tc.tile_pool(name="xg_p", bufs=2) as xg_p, \
             tc.tile_pool(name="h1_p", bufs=2) as h1_p, \
             tc.tile_pool(name="w1_p", bufs=3) as w1_p, \
             tc.tile_pool(name="w2_p", bufs=3) as w2_p, \
             tc.tile_pool(name="out_p", bufs=2) as out_p, \
             tc.tile_pool(name="sm_p", bufs=3) as sm_p, \
             tc.tile_pool(name="ps1_p", bufs=3, space="PSUM") as ps1_p, \
             tc.tile_pool(name="ps2_p", bufs=4, space="PSUM") as ps2_p:

            for u in range(UNITS):
                xu = xg_p.tile([128, KO, C], f32r, tag="xu")
                nc.sync.dma_start(xu[:], xg_d[u].bitcast(f32r))
                gu = sm_p.tile([128, CM], f32, tag="gu")
                nc.sync.dma_start(gu[:], gates_d[u])
                b1u = sm_p.tile([128, KO], f32, tag="b1u")
                nc.sync.dma_start(b1u[:], b1_d[u])

                w1r = w1_d[u].rearrange("(ko p) i -> p ko i", p=128)
                w2r = w2_d[u].rearrange("(ko p) h -> p ko h", p=128)

                h1 = h1_p.tile([128, KO, C], f32r, tag="h1")

                # ---- mm1: h1[i, c] = gelu(sum_h W1[h,i] * X^T[h,c] + b1[i])
                for ic in range(N_W1C):
                    w1c = w1_p.tile([128, KO, W1CW], f32r, tag="w1c")
                    nc.sync.dma_start(
                        w1c[:], w1r[:, :, ic * W1CW:(ic + 1) * W1CW].bitcast(f32r))
                    for s in range(W1CW // 128):
                        i_out = ic * (W1CW // 128) + s
                        ps = ps1_p.tile([128, C], mybir.dt.float32, tag="ps1")
                        for ko in range(KO):
                            nc.tensor.matmul(
                                ps[:],
                                w1c[:, ko, s * 128:(s + 1) * 128],
                                xu[:, ko, :],
                                start=(ko == 0),
                                stop=(ko == KO - 1),
                            )
                        nc.scalar.activation(
                            h1[:, i_out, :], ps[:], GELU,
                            bias=b1u[:, i_out:i_out + 1])

                # ---- mm2: y[c, h] = gate[c] * sum_i h1[i, c] * W2[i, h]
                ou = out_p.tile([128, CM, HID], f32, tag="ou")
                for hc in range(N_W2C):
                    w2c = w2_p.tile([128, KO, W2CW], f32r, tag="w2c")
                    nc.sync.dma_start(
                        w2c[:], w2r[:, :, hc * W2CW:(hc + 1) * W2CW].bitcast(f32r))
                    for ci in range(CM):
                        ps2 = ps2_p.tile([128, W2CW], mybir.dt.float32, tag="ps2")
                        for ko in range(KO):
                            nc.tensor.matmul(
                                ps2[:],
                                h1[:, ko, ci * 128:(ci + 1) * 128],
                                w2c[:, ko, :],
                                start=(ko == 0),
                                stop=(ko == KO - 1),
                            )
                        nc.vector.tensor_scalar_mul(
                            ou[:, ci, hc * W2CW:(hc + 1) * W2CW], ps2[:],
                            gu[:, ci:ci + 1])
                nc.sync.dma_start(out_d[u], ou[:])

    nc.compile()
    return nc


def _get_nc():
    if "nc" not in _cache:
        _cache["nc"] = _build_nc()
    return _cache["nc"]


def _gelu_np(v):
    from scipy.special import erf
    v = v.astype(np.float32)
    return (0.5 * v * (1.0 + erf(v / np.sqrt(2.0)))).astype(np.float32)


def kernel(x, w1_shared, b1_shared, w2_shared, b2_shared,
           router_w, router_b, w1, b1, w2, b2):
    from concourse.bass_utils import run_bass_kernel_spmd

    x = np.asarray(x, np.float32)
    w1 = np.asarray(w1, np.float32)
    b1 = np.asarray(b1, np.float32)
    w2 = np.asarray(w2, np.float32)
    b2 = np.asarray(b2, np.float32)
    w1_shared = np.asarray(w1_shared, np.float32)
    b1_shared = np.asarray(b1_shared, np.float32)
    w2_shared = np.asarray(w2_shared, np.float32)
    b2_shared = np.asarray(b2_shared, np.float32)
    router_w = np.asarray(router_w, np.float32)
    router_b = np.asarray(router_b, np.float32)

    xf = x.reshape(T, HID)

    # ---------------- host routing ----------------
    logits = xf @ router_w + router_b
    m = logits.max(-1, keepdims=True)
    ex = np.exp(logits - m, dtype=np.float32)
    affin = ex / ex.sum(-1, keepdims=True, dtype=np.float32)
    order = np.argsort(-affin, axis=-1, kind="stable")[:, :TOP_K]   # [T, K]
    vals = np.take_along_axis(affin, order, axis=-1)                # [T, K]

    # group (token, gate) pairs by expert
    flat_e = order.ravel()
    flat_t = np.repeat(np.arange(T), TOP_K)
    flat_g = vals.ravel()
    sort = np.argsort(flat_e, kind="stable")
    se, st, sg = flat_e[sort], flat_t[sort], flat_g[sort]
    starts = np.searchsorted(se, np.arange(E + 1))
    tok_by_e = [st[starts[e]:starts[e + 1]] for e in range(E)]
    gate_by_e = [sg[starts[e]:starts[e + 1]] for e in range(E)]

    # slot table: 64 expert slots; slot s -> (expert, token idx, gates)
    slot_expert = [s if s < E else -1 for s in range(NCORES * 8)]
    slot_tok = [tok_by_e[s][:C] if s < E else np.empty(0, np.int64)
                for s in range(NCORES * 8)]
    slot_gate = [gate_by_e[s][:C] if s < E else np.empty(0, np.float32)
                 for s in range(NCORES * 8)]

    # overflow: worst-overflowing expert spills into slot 63; the rest go to
    # an exact host fallback (rare for any randomly-routed input).
    host_fallback = []  # (expert, tokens, gates)
    over = [e for e in range(E) if len(tok_by_e[e]) > C]
    if over:
        over.sort(key=lambda e: len(tok_by_e[e]), reverse=True)
        e0 = over[0]
        slot_expert[E] = e0
        slot_tok[E] = tok_by_e[e0][C:2 * C]
        slot_gate[E] = gate_by_e[e0][C:2 * C]
        if len(tok_by_e[e0]) > 2 * C:
            host_fallback.append((e0, tok_by_e[e0][2 * C:], gate_by_e[e0][2 * C:]))
        for e in over[1:]:
            host_fallback.append((e, tok_by_e[e][C:], gate_by_e[e][C:]))

    # ---------------- build per-core device inputs ----------------
    # x transposed + partition-tiled: xT_t[ko, p, t] = x[t, ko*128+p]
    xT_t = np.ascontiguousarray(xf.T).reshape(KO, 128, T)

    in_maps = []
    for c in range(NCORES):
        xg = np.zeros((UNITS, 128, KO, C), np.float32)
        gates = np.zeros((UNITS, 128, CM), np.float32)
        w1u = np.zeros((UNITS, HID, I), np.float32)
        b1u = np.zeros((UNITS, 128, KO), np.float32)
        w2u = np.zeros((UNITS, I, HID), np.float32)
        for u in range(8):
            s = c * 8 + u
            e = slot_expert[s]
            if e < 0 or len(slot_tok[s]) == 0:
                continue
            n = len(slot_tok[s])
            idx = np.zeros(C, np.int64)
            idx[:n] = slot_tok[s]
            xg[u] = xT_t[:, :, idx].swapaxes(0, 1)
            g = np.zeros(C, np.float32)
            g[:n] = slot_gate[s]
            gates[u] = g.reshape(CM, 128).T
            w1u[u] = w1[e]
            b1u[u] = b1[e].reshape(KO, 128).T
            w2u[u] = w2[e]
        # shared-expert unit
        xg[8] = xT_t[:, :, c * TSH:(c + 1) * TSH].swapaxes(0, 1)
        gates[8] = 1.0
        w1u[8] = w1_shared[0]
        b1u[8] = b1_shared[0].reshape(KO, 128).T
        w2u[8] = w2_shared[0]
        in_maps.append({"xg": xg, "gates": gates, "w1": w1u, "b1": b1u, "w2": w2u})

    # ---------------- run on 8 cores ----------------
    nc = _get_nc()
    res = run_bass_kernel_spmd(nc, in_maps, core_ids=list(range(NCORES)))
    outs = [r["out"] for r in res.results]   # [UNITS, 128, CM, HID] each

    # ---------------- host unshard / scatter ----------------
    acc = np.zeros((T, HID), np.float32)     # shared + routed
    # shared expert (unit 8 on each core), gate 1, + b2_shared
    for c in range(NCORES):
        ys = outs[c][8].transpose(1, 0, 2).reshape(TSH, HID)
        acc[c * TSH:(c + 1) * TSH] = ys + b2_shared[0]
    # routed experts: add gated slot outputs + gate * b2[e]
    for s in range(NCORES * 8):
        e = slot_expert[s]
        n = len(slot_tok[s])
        if e < 0 or n == 0:
            continue
        ye = outs[s // 8][s % 8].transpose(1, 0, 2).reshape(C, HID)[:n]
        # token indices are unique within one slot, so fancy += is safe
        acc[slot_tok[s]] += ye + slot_gate[s][:, None] * b2[e][None, :]
    # exact host fallback for overflow beyond device capacity
    for e, toks, gs in host_fallback:
        h = _gelu_np(xf[toks] @ w1[e] + b1[e])
        acc[toks] += gs[:, None] * (h @ w2[e] + b2[e])

    return (acc + xf).reshape(B, S, HID).astype(np.float32)


# revision 4
# speedup vs baseline: 1.2448x; 1.2448x over previous
"""MoE (63 routed experts, top-7, 1 shared expert) Trainium2 Bass kernel.

Strategy (expert parallelism, per sharding hint):
  - Host: router matmul + softmax + top-k (tiny: 0.7 GFLOP vs 220 GFLOP of
    expert FFNs), token gather per expert.
  - Device (8 NeuronCores, SPMD): each core runs 9 "units" of identical
    shape: 8 routed-expert slots (64 slots globally = 63 experts + 1
    overflow slot) and 1 shared-expert slot over a 1/8 token slice.
    Each unit: h = gelu(XeT^T @ W1 + b1); y = gate * (h @ W2), with
    full-rate matmuls (float32r or bf16), GELU fused into the PSUM
    eviction on the scalar engine, gating fused into the PSUM eviction on
    the vector engine.  Weights are host-pretiled into chunk-contiguous
    layout so every DMA is a flat [128 x bytes] block.
  - Host: scatter-add gated expert outputs (+ gate*b2), add shared out,
    bias and residual.

Capacity C=512 tokens/slot (seed-0 max expert load is 520; the single
largest-overflow expert spills into the overflow slot; anything beyond that
falls back to an exact host-side FFN for the few excess tokens).
"""

import os

import numpy as np

B, S, HID = 2, 2048, 1280
E = 63
I = 1280
TOP_K = 7
NCORES = 8
UNITS = 9          # 8 expert slots + 1 shared-expert slot
C = 512            # token capacity per expert slot
CM = C // 128      # 4
KO = HID // 128    # 10 contraction chunks
T = B * S          # 4096
TSH = T // NCORES  # 512 shared-expert tokens per core

W1CW = 256          # w1 chunk width along I (2 lhsT column groups)
W2CW = 320          # w2 chunk width along H (psum free dim for mm2)
N_W1C = I // W1CW   # 5
N_W2C = HID // W2CW  # 4

# "f32r": fp32 data, full-rate float32r matmuls (most accurate).
# "bf16": bf16 weights+activations, fp32 accumulate (halves DMA traffic).
WORK_DTYPE = os.environ.get("MOE_WDT", "bf16")

_cache = {}


def _build_nc(wdt):
    import concourse.mybir as mybir
    import concourse.tile as tile
    from concourse import bacc

    f32 = mybir.dt.float32
    GELU = mybir.ActivationFunctionType.Gelu
    if wdt == "f32r":
        mdt = mybir.dt.float32r
        ddt = f32    # dram dtype for weight/activation tensors
    else:
        mdt = mybir.dt.bfloat16
        ddt = mybir.dt.bfloat16

    nc = bacc.Bacc(None, target_bir_lowering=False)

    xg_d = nc.dram_tensor("xg", [UNITS, 128, KO, C], ddt, kind="ExternalInput")
    gates_d = nc.dram_tensor("gates", [UNITS, 128, CM], f32, kind="ExternalInput")
    w1_d = nc.dram_tensor("w1", [UNITS, N_W1C, 128, KO, W1CW], ddt,
                          kind="ExternalInput")
    b1_d = nc.dram_tensor("b1", [UNITS, 128, KO], f32, kind="ExternalInput")
    w2_d = nc.dram_tensor("w2", [UNITS, N_W2C, 128, KO, W2CW], ddt,
                          kind="ExternalInput")
    out_d = nc.dram_tensor("out", [UNITS, 128, CM, HID], f32, kind="ExternalOutput")

    def cast(ap):
        return ap.bitcast(mdt) if wdt == "f32r" else ap

    with tile.TileContext(nc) as tc:
        with tc.tile_pool(name="xg_p", bufs=2) as xg_p, \
             tc.tile_pool(name="h1_p", bufs=2) as h1_p, \
             tc.tile_pool(name="w1_p", bufs=3) as w1_p, \
             tc.tile_pool(name="w2_p", bufs=3) as w2_p, \
             tc.tile_pool(name="out_p", bufs=2) as out_p, \
             tc.tile_pool(name="sm_p", bufs=3) as sm_p, \
             tc.tile_pool(name="ps1_p", bufs=3, space="PSUM") as ps1_p, \
             tc.tile_pool(name="ps2_p", bufs=4, space="PSUM") as ps2_p:

            for u in range(UNITS):
                xu = xg_p.tile([128, KO, C], mdt, tag="xu")
                nc.sync.dma_start(xu[:], cast(xg_d[u]))
                gu = sm_p.tile([128, CM], f32, tag="gu")
                nc.sync.dma_start(gu[:], gates_d[u])
                b1u = sm_p.tile([128, KO], f32, tag="b1u")
                nc.sync.dma_start(b1u[:], b1_d[u])

                h1 = h1_p.tile([128, KO, C], mdt, tag="h1")

                # ---- mm1: h1[i, c] = gelu(sum_h W1[h,i] * X^T[h,c] + b1[i])
                for ic in range(N_W1C):
                    w1c = w1_p.tile([128, KO, W1CW], mdt, tag="w1c")
                    nc.sync.dma_start(w1c[:], cast(w1_d[u, ic]))
                    for s in range(W1CW // 128):
                        i_out = ic * (W1CW // 128) + s
                        ps = ps1_p.tile([128, C], f32, tag="ps1")
                        for ko in range(KO):
                            nc.tensor.matmul(
                                ps[:],
                                w1c[:, ko, s * 128:(s + 1) * 128],
                                xu[:, ko, :],
                                start=(ko == 0),
                                stop=(ko == KO - 1),
                            )
                        nc.scalar.activation(
                            h1[:, i_out, :], ps[:], GELU,
                            bias=b1u[:, i_out:i_out + 1])

                # ---- mm2: y[c, h] = gate[c] * sum_i h1[i, c] * W2[i, h]
                ou = out_p.tile([128, CM, HID], f32, tag="ou")
                for hc in range(N_W2C):
                    w2c = w2_p.tile([128, KO, W2CW], mdt, tag="w2c")
                    nc.sync.dma_start(w2c[:], cast(w2_d[u, hc]))
                    for ci in range(CM):
                        ps2 = ps2_p.tile([128, W2CW], f32, tag="ps2")
                        for ko in range(KO):
                            nc.tensor.matmul(
                                ps2[:],
                                h1[:, ko, ci * 128:(ci + 1) * 128],
                                w2c[:, ko, :],
                                start=(ko == 0),
                                stop=(ko == KO - 1),
                            )
                        nc.vector.tensor_scalar_mul(
                            ou[:, ci, hc * W2CW:(hc + 1) * W2CW], ps2[:],
                            gu[:, ci:ci + 1])
                nc.sync.dma_start(out_d[u], ou[:])

    nc.compile()
    return nc


def _get_nc(wdt):
    if wdt not in _cache:
        _cache[wdt] = _build_nc(wdt)
    return _cache[wdt]


def _np_wdt(wdt):
    if wdt == "bf16":
        import ml_dtypes
        return np.dtype(ml_dtypes.bfloat16)
    return np.dtype(np.float32)


def _gelu_np(v):
    from scipy.special import erf
    v = v.astype(np.float32)
    return (0.5 * v * (1.0 + erf(v / np.sqrt(2.0)))).astype(np.float32)


def _tile_w1(w):
    # [H, I] -> [N_W1C, 128, KO, W1CW] with w1t[ic, p, ko, j] = w[ko*128+p, ic*W1CW+j]
    return w.reshape(KO, 128, N_W1C, W1CW).transpose(2, 1, 0, 3)


def _tile_w2(w):
    # [I, H] -> [N_W2C, 128, KO, W2CW]
    return w.reshape(KO, 128, N_W2C, W2CW).transpose(2, 1, 0, 3)


def kernel(x, w1_shared, b1_shared, w2_shared, b2_shared,
           router_w, router_b, w1, b1, w2, b2):
    from concourse.bass_utils import run_bass_kernel_spmd

    wdt = WORK_DTYPE
    ndt = _np_wdt(wdt)

    x = np.asarray(x, np.float32)
    w1 = np.asarray(w1, np.float32)
    b1 = np.asarray(b1, np.float32)
    w2 = np.asarray(w2, np.float32)
    b2 = np.asarray(b2, np.float32)
    w1_shared = np.asarray(w1_shared, np.float32)
    b1_shared = np.asarray(b1_shared, np.float32)
    w2_shared = np.asarray(w2_shared, np.float32)
    b2_shared = np.asarray(b2_shared, np.float32)
    router_w = np.asarray(router_w, np.float32)
    router_b = np.asarray(router_b, np.float32)

    xf = x.reshape(T, HID)

    # ---------------- host routing ----------------
    logits = xf @ router_w + router_b
    m = logits.max(-1, keepdims=True)
    ex = np.exp(logits - m, dtype=np.float32)
    affin = ex / ex.sum(-1, keepdims=True, dtype=np.float32)
    order = np.argsort(-affin, axis=-1, kind="stable")[:, :TOP_K]   # [T, K]
    vals = np.take_along_axis(affin, order, axis=-1)                # [T, K]

    # group (token, gate) pairs by expert
    flat_e = order.ravel()
    flat_t = np.repeat(np.arange(T), TOP_K)
    flat_g = vals.ravel()
    sort = np.argsort(flat_e, kind="stable")
    se, st, sg = flat_e[sort], flat_t[sort], flat_g[sort]
    starts = np.searchsorted(se, np.arange(E + 1))
    tok_by_e = [st[starts[e]:starts[e + 1]] for e in range(E)]
    gate_by_e = [sg[starts[e]:starts[e + 1]] for e in range(E)]

    # slot table: 64 expert slots; slot s -> (expert, token idx, gates)
    slot_expert = [s if s < E else -1 for s in range(NCORES * 8)]
    slot_tok = [tok_by_e[s][:C] if s < E else np.empty(0, np.int64)
                for s in range(NCORES * 8)]
    slot_gate = [gate_by_e[s][:C] if s < E else np.empty(0, np.float32)
                 for s in range(NCORES * 8)]

    # overflow: worst-overflowing expert spills into slot 63; the rest go to
    # an exact host fallback (rare for any randomly-routed input).
    host_fallback = []  # (expert, tokens, gates)
    over = [e for e in range(E) if len(tok_by_e[e]) > C]
    if over:
        over.sort(key=lambda e: len(tok_by_e[e]), reverse=True)
        e0 = over[0]
        slot_expert[E] = e0
        slot_tok[E] = tok_by_e[e0][C:2 * C]
        slot_gate[E] = gate_by_e[e0][C:2 * C]
        if len(tok_by_e[e0]) > 2 * C:
            host_fallback.append((e0, tok_by_e[e0][2 * C:], gate_by_e[e0][2 * C:]))
        for e in over[1:]:
            host_fallback.append((e, tok_by_e[e][C:], gate_by_e[e][C:]))

    # ---------------- build per-core device inputs ----------------
    # x transposed + partition-tiled: xT_t[ko, p, t] = x[t, ko*128+p]
    xT_t = np.ascontiguousarray(xf.T).astype(ndt).reshape(KO, 128, T)

    w1t_sh = _tile_w1(w1_shared[0]).astype(ndt)
    w2t_sh = _tile_w2(w2_shared[0]).astype(ndt)
    b1t_sh = b1_shared[0].reshape(KO, 128).T

    in_maps = []
    for c in range(NCORES):
        xg = np.zeros((UNITS, 128, KO, C), ndt)
        gates = np.zeros((UNITS, 128, CM), np.float32)
        w1u = np.zeros((UNITS, N_W1C, 128, KO, W1CW), ndt)
        b1u = np.zeros((UNITS, 128, KO), np.float32)
        w2u = np.zeros((UNITS, N_W2C, 128, KO, W2CW), ndt)
        for u in range(8):
            s = c * 8 + u
            e = slot_expert[s]
            if e < 0 or len(slot_tok[s]) == 0:
                continue
            n = len(slot_tok[s])
            idx = np.zeros(C, np.int64)
            idx[:n] = slot_tok[s]
            xg[u] = xT_t[:, :, idx].swapaxes(0, 1)
            g = np.zeros(C, np.float32)
            g[:n] = slot_gate[s]
            gates[u] = g.reshape(CM, 128).T
            w1u[u] = _tile_w1(w1[e]).astype(ndt)
            b1u[u] = b1[e].reshape(KO, 128).T
            w2u[u] = _tile_w2(w2[e]).astype(ndt)
        # shared-expert unit
        xg[8] = xT_t[:, :, c * TSH:(c + 1) * TSH].swapaxes(0, 1)
        gates[8] = 1.0
        w1u[8] = w1t_sh
        b1u[8] = b1t_sh
        w2u[8] = w2t_sh
        in_maps.append({"xg": xg, "gates": gates, "w1": w1u, "b1": b1u, "w2": w2u})

    # ---------------- run on 8 cores ----------------
    nc = _get_nc(wdt)
    res = run_bass_kernel_spmd(nc, in_maps, core_ids=list(range(NCORES)))
    outs = [r["out"] for r in res.results]   # [UNITS, 128, CM, HID] each

    # ---------------- host unshard / scatter ----------------
    acc = np.zeros((T, HID), np.float32)     # shared + routed
    # shared expert (unit 8 on each core), gate 1, + b2_shared
    for c in range(NCORES):
        ys = outs[c][8].transpose(1, 0, 2).reshape(TSH, HID)
        acc[c * TSH:(c + 1) * TSH] = ys + b2_shared[0]
    # routed experts: add gated slot outputs + gate * b2[e]
    for s in range(NCORES * 8):
        e = slot_expert[s]
        n = len(slot_tok[s])
        if e < 0 or n == 0:
            continue
        ye = outs[s // 8][s % 8].transpose(1, 0, 2).reshape(C, HID)[:n]
        # token indices are unique within one slot, so fancy += is safe
        acc[slot_tok[s]] += ye + slot_gate[s][:, None] * b2[e][None, :]
    # exact host fallback for overflow beyond device capacity
    for e, toks, gs in host_fallback:
        h = _gelu_np(xf[toks] @ w1[e] + b1[e])
        acc[toks] += gs[:, None] * (h @ w2[e] + b2[e])

    return (acc + xf).reshape(B, S, HID).astype(np.float32)


# revision 8
# speedup vs baseline: 1.2968x; 1.0417x over previous
"""MoE (63 routed experts, top-7, 1 shared expert) Trainium2 Bass kernel.

Strategy (expert parallelism, per sharding hint):
  - Host: router matmul + softmax + top-k (tiny: 0.7 GFLOP vs 220 GFLOP of
    expert FFNs), token gather per expert.
  - Device (8 NeuronCores, SPMD): each core runs 9 "units" of identical
    shape: 8 routed-expert slots (64 slots globally = 63 experts + 1
    overflow slot) and 1 shared-expert slot over a 1/8 token slice.
    Each unit: h = gelu(XeT^T @ W1 + b1); y = gate * (h @ W2), with
    full-rate matmuls (float32r or bf16), GELU fused into the PSUM
    eviction on the scalar engine, gating fused into the PSUM eviction on
    the vector engine.  Weights are host-pretiled into chunk-contiguous
    layout so every DMA is a flat [128 x bytes] block.
  - Host: scatter-add gated expert outputs (+ gate*b2), add shared out,
    bias and residual.

Capacity C=512 tokens/slot (seed-0 max expert load is 520; the single
largest-overflow expert spills into the overflow slot; anything beyond that
falls back to an exact host-side FFN for the few excess tokens).
"""

import os

import numpy as np

B, S, HID = 2, 2048, 1280
E = 63
I = 1280
TOP_K = 7
NCORES = 8
UNITS = 9          # 8 expert slots + 1 shared-expert slot
C = 512            # token capacity per expert slot
CM = C // 128      # 4
KO = HID // 128    # 10 contraction chunks
T = B * S          # 4096
TSH = T // NCORES  # 512 shared-expert tokens per core

W1CW = 256          # w1 chunk width along I (2 lhsT column groups)
W2CW = 320          # w2 chunk width along H (psum free dim for mm2)
N_W1C = I // W1CW   # 5
N_W2C = HID // W2CW  # 4

# "f32r": fp32 data, full-rate float32r matmuls (most accurate).
# "bf16": bf16 weights+activations, fp32 accumulate (halves DMA traffic).
# "fp16": like bf16 but 4x finer mantissa; all values here are well within
#         fp16 range, so this is strictly more accurate at the same speed.
WORK_DTYPE = os.environ.get("MOE_WDT", "fp16")

_cache = {}


def _build_nc(wdt):
    import concourse.mybir as mybir
    import concourse.tile as tile
    from concourse import bacc

    f32 = mybir.dt.float32
    GELU = mybir.ActivationFunctionType.Gelu
    if wdt == "f32r":
        mdt = mybir.dt.float32r
        ddt = f32    # dram dtype for weight/activation tensors
        bufs = dict(xu=2, h1=2, w1c=3, w2c=3, ou=2)
    else:
        mdt = mybir.dt.float16 if wdt == "fp16" else mybir.dt.bfloat16
        ddt = mdt
        bufs = dict(xu=3, h1=3, w1c=4, w2c=4, ou=3)

    nc = bacc.Bacc(None, target_bir_lowering=False)

    xg_d = nc.dram_tensor("xg", [UNITS, 128, KO, C], ddt, kind="ExternalInput")
    gates_d = nc.dram_tensor("gates", [UNITS, 128, CM], f32, kind="ExternalInput")
    w1_d = nc.dram_tensor("w1", [UNITS, N_W1C, 128, KO, W1CW], ddt,
                          kind="ExternalInput")
    b1_d = nc.dram_tensor("b1", [UNITS, 128, KO], f32, kind="ExternalInput")
    w2_d = nc.dram_tensor("w2", [UNITS, N_W2C, 128, KO, W2CW], ddt,
                          kind="ExternalInput")
    out_d = nc.dram_tensor("out", [UNITS, 128, CM, HID], f32, kind="ExternalOutput")

    def cast(ap):
        return ap.bitcast(mdt) if wdt == "f32r" else ap

    with tile.TileContext(nc) as tc:
        with tc.tile_pool(name="xg_p", bufs=bufs["xu"]) as xg_p, \
             tc.tile_pool(name="h1_p", bufs=bufs["h1"]) as h1_p, \
             tc.tile_pool(name="w1_p", bufs=bufs["w1c"]) as w1_p, \
             tc.tile_pool(name="w2_p", bufs=bufs["w2c"]) as w2_p, \
             tc.tile_pool(name="out_p", bufs=bufs["ou"]) as out_p, \
             tc.tile_pool(name="sm_p", bufs=3) as sm_p, \
             tc.tile_pool(name="ps1_p", bufs=3, space="PSUM") as ps1_p, \
             tc.tile_pool(name="ps2_p", bufs=4, space="PSUM") as ps2_p:

            for u in range(UNITS):
                xu = xg_p.tile([128, KO, C], mdt, tag="xu")
                nc.sync.dma_start(xu[:], cast(xg_d[u]))
                gu = sm_p.tile([128, CM], f32, tag="gu")
                nc.sync.dma_start(gu[:], gates_d[u])
                b1u = sm_p.tile([128, KO], f32, tag="b1u")
                nc.sync.dma_start(b1u[:], b1_d[u])

                h1 = h1_p.tile([128, KO, C], mdt, tag="h1")

                # ---- mm1: h1[i, c] = gelu(sum_h W1[h,i] * X^T[h,c] + b1[i])
                for ic in range(N_W1C):
                    w1c = w1_p.tile([128, KO, W1CW], mdt, tag="w1c")
                    nc.sync.dma_start(w1c[:], cast(w1_d[u, ic]))
                    for s in range(W1CW // 128):
                        i_out = ic * (W1CW // 128) + s
                        ps = ps1_p.tile([128, C], f32, tag="ps1")
                        for ko in range(KO):
                            nc.tensor.matmul(
                                ps[:],
                                w1c[:, ko, s * 128:(s + 1) * 128],
                                xu[:, ko, :],
                                start=(ko == 0),
                                stop=(ko == KO - 1),
                            )
                        nc.scalar.activation(
                            h1[:, i_out, :], ps[:], GELU,
                            bias=b1u[:, i_out:i_out + 1])

                # ---- mm2: y[c, h] = gate[c] * sum_i h1[i, c] * W2[i, h]
                ou = out_p.tile([128, CM, HID], f32, tag="ou")
                for hc in range(N_W2C):
                    w2c = w2_p.tile([128, KO, W2CW], mdt, tag="w2c")
                    nc.sync.dma_start(w2c[:], cast(w2_d[u, hc]))
                    for ci in range(CM):
                        ps2 = ps2_p.tile([128, W2CW], f32, tag="ps2")
                        for ko in range(KO):
                            nc.tensor.matmul(
                                ps2[:],
                                h1[:, ko, ci * 128:(ci + 1) * 128],
                                w2c[:, ko, :],
                                start=(ko == 0),
                                stop=(ko == KO - 1),
                            )
                        nc.vector.tensor_scalar_mul(
                            ou[:, ci, hc * W2CW:(hc + 1) * W2CW], ps2[:],
                            gu[:, ci:ci + 1])
                nc.sync.dma_start(out_d[u], ou[:])

    nc.compile()
    return nc


def _get_nc(wdt):
    if wdt not in _cache:
        _cache[wdt] = _build_nc(wdt)
    return _cache[wdt]


def _np_wdt(wdt):
    if wdt == "bf16":
        import ml_dtypes
        return np.dtype(ml_dtypes.bfloat16)
    if wdt == "fp16":
        return np.dtype(np.float16)
    return np.dtype(np.float32)


def _gelu_np(v):
    from scipy.special import erf
    v = v.astype(np.float32)
    return (0.5 * v * (1.0 + erf(v / np.sqrt(2.0)))).astype(np.float32)


def _tile_w1(w):
    # [H, I] -> [N_W1C, 128, KO, W1CW] with w1t[ic, p, ko, j] = w[ko*128+p, ic*W1CW+j]
    return w.reshape(KO, 128, N_W1C, W1CW).transpose(2, 1, 0, 3)


def _tile_w2(w):
    # [I, H] -> [N_W2C, 128, KO, W2CW]
    return w.reshape(KO, 128, N_W2C, W2CW).transpose(2, 1, 0, 3)


def kernel(x, w1_shared, b1_shared, w2_shared, b2_shared,
           router_w, router_b, w1, b1, w2, b2):
    from concourse.bass_utils import run_bass_kernel_spmd

    wdt = WORK_DTYPE
    ndt = _np_wdt(wdt)

    x = np.asarray(x, np.float32)
    w1 = np.asarray(w1, np.float32)
    b1 = np.asarray(b1, np.float32)
    w2 = np.asarray(w2, np.float32)
    b2 = np.asarray(b2, np.float32)
    w1_shared = np.asarray(w1_shared, np.float32)
    b1_shared = np.asarray(b1_shared, np.float32)
    w2_shared = np.asarray(w2_shared, np.float32)
    b2_shared = np.asarray(b2_shared, np.float32)
    router_w = np.asarray(router_w, np.float32)
    router_b = np.asarray(router_b, np.float32)

    xf = x.reshape(T, HID)

    # ---------------- host routing ----------------
    logits = xf @ router_w + router_b
    m = logits.max(-1, keepdims=True)
    ex = np.exp(logits - m, dtype=np.float32)
    affin = ex / ex.sum(-1, keepdims=True, dtype=np.float32)
    order = np.argsort(-affin, axis=-1, kind="stable")[:, :TOP_K]   # [T, K]
    vals = np.take_along_axis(affin, order, axis=-1)                # [T, K]

    # group (token, gate) pairs by expert
    flat_e = order.ravel()
    flat_t = np.repeat(np.arange(T), TOP_K)
    flat_g = vals.ravel()
    sort = np.argsort(flat_e, kind="stable")
    se, st, sg = flat_e[sort], flat_t[sort], flat_g[sort]
    starts = np.searchsorted(se, np.arange(E + 1))
    tok_by_e = [st[starts[e]:starts[e + 1]] for e in range(E)]
    gate_by_e = [sg[starts[e]:starts[e + 1]] for e in range(E)]

    # slot table: 64 expert slots; slot s -> (expert, token idx, gates)
    slot_expert = [s if s < E else -1 for s in range(NCORES * 8)]
    slot_tok = [tok_by_e[s][:C] if s < E else np.empty(0, np.int64)
                for s in range(NCORES * 8)]
    slot_gate = [gate_by_e[s][:C] if s < E else np.empty(0, np.float32)
                 for s in range(NCORES * 8)]

    # overflow: worst-overflowing expert spills into slot 63; the rest go to
    # an exact host fallback (rare for any randomly-routed input).
    host_fallback = []  # (expert, tokens, gates)
    over = [e for e in range(E) if len(tok_by_e[e]) > C]
    if over:
        over.sort(key=lambda e: len(tok_by_e[e]), reverse=True)
        e0 = over[0]
        slot_expert[E] = e0
        slot_tok[E] = tok_by_e[e0][C:2 * C]
        slot_gate[E] = gate_by_e[e0][C:2 * C]
        if len(tok_by_e[e0]) > 2 * C:
            host_fallback.append((e0, tok_by_e[e0][2 * C:], gate_by_e[e0][2 * C:]))
        for e in over[1:]:
            host_fallback.append((e, tok_by_e[e][C:], gate_by_e[e][C:]))

    # ---------------- build per-core device inputs ----------------
    # x transposed + partition-tiled: xT_t[ko, p, t] = x[t, ko*128+p]
    xT_t = np.ascontiguousarray(xf.T).astype(ndt).reshape(KO, 128, T)

    w1t_sh = _tile_w1(w1_shared[0]).astype(ndt)
    w2t_sh = _tile_w2(w2_shared[0]).astype(ndt)
    b1t_sh = b1_shared[0].reshape(KO, 128).T

    in_maps = []
    for c in range(NCORES):
        xg = np.zeros((UNITS, 128, KO, C), ndt)
        gates = np.zeros((UNITS, 128, CM), np.float32)
        w1u = np.zeros((UNITS, N_W1C, 128, KO, W1CW), ndt)
        b1u = np.zeros((UNITS, 128, KO), np.float32)
        w2u = np.zeros((UNITS, N_W2C, 128, KO, W2CW), ndt)
        for u in range(8):
            s = c * 8 + u
            e = slot_expert[s]
            if e < 0 or len(slot_tok[s]) == 0:
                continue
            n = len(slot_tok[s])
            idx = np.zeros(C, np.int64)
            idx[:n] = slot_tok[s]
            xg[u] = xT_t[:, :, idx].swapaxes(0, 1)
            g = np.zeros(C, np.float32)
            g[:n] = slot_gate[s]
            gates[u] = g.reshape(CM, 128).T
            w1u[u] = _tile_w1(w1[e]).astype(ndt)
            b1u[u] = b1[e].reshape(KO, 128).T
            w2u[u] = _tile_w2(w2[e]).astype(ndt)
        # shared-expert unit
        xg[8] = xT_t[:, :, c * TSH:(c + 1) * TSH].swapaxes(0, 1)
        gates[8] = 1.0
        w1u[8] = w1t_sh
        b1u[8] = b1t_sh
        w2u[8] = w2t_sh
        in_maps.append({"xg": xg, "gates": gates, "w1": w1u, "b1": b1u, "w2": w2u})

    # ---------------- run on 8 cores ----------------
    nc = _get_nc(wdt)
    res = run_bass_kernel_spmd(nc, in_maps, core_ids=list(range(NCORES)))
    outs = [r["out"] for r in res.results]   # [UNITS, 128, CM, HID] each

    # ---------------- host unshard / scatter ----------------
    acc = np.zeros((T, HID), np.float32)     # shared + routed
    # shared expert (unit 8 on each core), gate 1, + b2_shared
    for c in range(NCORES):
        ys = outs[c][8].transpose(1, 0, 2).reshape(TSH, HID)
        acc[c * TSH:(c + 1) * TSH] = ys + b2_shared[0]
    # routed experts: add gated slot outputs + gate * b2[e]
    for s in range(NCORES * 8):
        e = slot_expert[s]
        n = len(slot_tok[s])
        if e < 0 or n == 0:
            continue
        ye = outs[s // 8][s % 8].transpose(1, 0, 2).reshape(C, HID)[:n]
        # token indices are unique within one slot, so fancy += is safe
        acc[slot_tok[s]] += ye + slot_gate[s][:, None] * b2[e][None, :]
    # exact host fallback for overflow beyond device capacity
    for e, toks, gs in host_fallback:
        h = _gelu_np(xf[toks] @ w1[e] + b1[e])
        acc[toks] += gs[:, None] * (h @ w2[e] + b2[e])

    return (acc + xf).reshape(B, S, HID).astype(np.float32)


# revision 11
# speedup vs baseline: 1.3069x; 1.0078x over previous
"""MoE (63 routed experts, top-7, 1 shared expert) Trainium2 Bass kernel.

Strategy (expert parallelism, per sharding hint):
  - Host: router matmul + softmax + top-k (tiny: 0.7 GFLOP vs 220 GFLOP of
    expert FFNs), token gather per expert.
  - Device (8 NeuronCores, SPMD): each core runs 9 "units" of identical
    shape: 8 routed-expert slots (64 slots globally = 63 experts + 1
    overflow slot) and 1 shared-expert slot over a 1/8 token slice.
    Each unit: h = gelu(XeT^T @ W1 + b1); y = gate * (h @ W2), with
    full-rate matmuls (float32r or bf16), GELU fused into the PSUM
    eviction on the scalar engine, gating fused into the PSUM eviction on
    the vector engine.  Weights are host-pretiled into chunk-contiguous
    layout so every DMA is a flat [128 x bytes] block.
  - Host: scatter-add gated expert outputs (+ gate*b2), add shared out,
    bias and residual.

Capacity C=512 tokens/slot (seed-0 max expert load is 520; the single
largest-overflow expert spills into the overflow slot; anything beyond that
falls back to an exact host-side FFN for the few excess tokens).
"""

import os

import numpy as np

B, S, HID = 2, 2048, 1280
E = 63
I = 1280
TOP_K = 7
NCORES = 8
UNITS = 9          # 8 expert slots + 1 shared-expert slot
C = 512            # token capacity per expert slot
CM = C // 128      # 4
KO = HID // 128    # 10 contraction chunks
T = B * S          # 4096
TSH = T // NCORES  # 512 shared-expert tokens per core

W1CW = 256          # w1 chunk width along I (2 lhsT column groups)
W2CW = 320          # w2 chunk width along H (psum free dim for mm2)
N_W1C = I // W1CW   # 5
N_W2C = HID // W2CW  # 4

# "f32r": fp32 data, full-rate float32r matmuls (most accurate).
# "bf16": bf16 weights+activations, fp32 accumulate (halves DMA traffic).
# "fp16": like bf16 but 4x finer mantissa; all values here are well within
#         fp16 range, so this is strictly more accurate at the same speed.
WORK_DTYPE = os.environ.get("MOE_WDT", "fp16")

_cache = {}


def _build_nc(wdt):
    import concourse.mybir as mybir
    import concourse.tile as tile
    from concourse import bacc

    f32 = mybir.dt.float32
    GELU = mybir.ActivationFunctionType.Gelu
    if wdt == "f32r":
        mdt = mybir.dt.float32r
        ddt = f32    # dram dtype for weight/activation tensors
        bufs = dict(xu=2, h1=2, w1c=3, w2c=3, ou=2)
    else:
        mdt = mybir.dt.float16 if wdt == "fp16" else mybir.dt.bfloat16
        ddt = mdt
        bufs = dict(xu=3, h1=3, w1c=4, w2c=6, ou=6)

    nc = bacc.Bacc(None, target_bir_lowering=False)

    xg_d = nc.dram_tensor("xg", [UNITS, 128, KO, C], ddt, kind="ExternalInput")
    gates_d = nc.dram_tensor("gates", [UNITS, 128, CM], f32, kind="ExternalInput")
    w1_d = nc.dram_tensor("w1", [UNITS, N_W1C, 128, KO, W1CW], ddt,
                          kind="ExternalInput")
    b1_d = nc.dram_tensor("b1", [UNITS, 128, KO], f32, kind="ExternalInput")
    w2_d = nc.dram_tensor("w2", [UNITS, N_W2C, 128, KO, W2CW], ddt,
                          kind="ExternalInput")
    out_d = nc.dram_tensor("out", [UNITS, 128, CM, HID], f32, kind="ExternalOutput")

    def cast(ap):
        return ap.bitcast(mdt) if wdt == "f32r" else ap

    with tile.TileContext(nc) as tc:
        with tc.tile_pool(name="xg_p", bufs=bufs["xu"]) as xg_p, \
             tc.tile_pool(name="h1_p", bufs=bufs["h1"]) as h1_p, \
             tc.tile_pool(name="w1_p", bufs=bufs["w1c"]) as w1_p, \
             tc.tile_pool(name="w2_p", bufs=bufs["w2c"]) as w2_p, \
             tc.tile_pool(name="out_p", bufs=bufs["ou"]) as out_p, \
             tc.tile_pool(name="sm_p", bufs=3) as sm_p, \
             tc.tile_pool(name="ps1_p", bufs=3, space="PSUM") as ps1_p, \
             tc.tile_pool(name="ps2_p", bufs=4, space="PSUM") as ps2_p:

            for u in range(UNITS):
                xu = xg_p.tile([128, KO, C], mdt, tag="xu")
                # split halves so the first matmuls can start sooner
                nc.sync.dma_start(xu[:, :KO // 2, :], cast(xg_d[u, :, :KO // 2, :]))
                nc.sync.dma_start(xu[:, KO // 2:, :], cast(xg_d[u, :, KO // 2:, :]))
                gu = sm_p.tile([128, CM], f32, tag="gu")
                nc.sync.dma_start(gu[:], gates_d[u])
                b1u = sm_p.tile([128, KO], f32, tag="b1u")
                nc.sync.dma_start(b1u[:], b1_d[u])

                h1 = h1_p.tile([128, KO, C], mdt, tag="h1")

                # ---- mm1: h1[i, c] = gelu(sum_h W1[h,i] * X^T[h,c] + b1[i])
                for ic in range(N_W1C):
                    w1c = w1_p.tile([128, KO, W1CW], mdt, tag="w1c")
                    nc.sync.dma_start(w1c[:], cast(w1_d[u, ic]))
                    for s in range(W1CW // 128):
                        i_out = ic * (W1CW // 128) + s
                        ps = ps1_p.tile([128, C], f32, tag="ps1")
                        for ko in range(KO):
                            nc.tensor.matmul(
                                ps[:],
                                w1c[:, ko, s * 128:(s + 1) * 128],
                                xu[:, ko, :],
                                start=(ko == 0),
                                stop=(ko == KO - 1),
                            )
                        nc.scalar.activation(
                            h1[:, i_out, :], ps[:], GELU,
                            bias=b1u[:, i_out:i_out + 1])

                # ---- mm2: y[c, h] = gate[c] * sum_i h1[i, c] * W2[i, h]
                w2cs = []
                for hc in range(N_W2C):
                    w2c = w2_p.tile([128, KO, W2CW], mdt, tag="w2c")
                    nc.sync.dma_start(w2c[:], cast(w2_d[u, hc]))
                    w2cs.append(w2c)
                for ci in range(CM):
                    ou = out_p.tile([128, HID], f32, tag="ou")
                    for hc in range(N_W2C):
                        ps2 = ps2_p.tile([128, W2CW], f32, tag="ps2")
                        for ko in range(KO):
                            nc.tensor.matmul(
                                ps2[:],
                                h1[:, ko, ci * 128:(ci + 1) * 128],
                                w2cs[hc][:, ko, :],
                                start=(ko == 0),
                                stop=(ko == KO - 1),
                            )
                        nc.vector.tensor_scalar_mul(
                            ou[:, hc * W2CW:(hc + 1) * W2CW], ps2[:],
                            gu[:, ci:ci + 1])
                    nc.sync.dma_start(out_d[u, :, ci, :], ou[:])

    nc.compile()
    return nc


def _get_nc(wdt):
    if wdt not in _cache:
        _cache[wdt] = _build_nc(wdt)
    return _cache[wdt]


def _np_wdt(wdt):
    if wdt == "bf16":
        import ml_dtypes
        return np.dtype(ml_dtypes.bfloat16)
    if wdt == "fp16":
        return np.dtype(np.float16)
    return np.dtype(np.float32)


def _gelu_np(v):
    from scipy.special import erf
    v = v.astype(np.float32)
    return (0.5 * v * (1.0 + erf(v / np.sqrt(2.0)))).astype(np.float32)


def _tile_w1(w):
    # [H, I] -> [N_W1C, 128, KO, W1CW] with w1t[ic, p, ko, j] = w[ko*128+p, ic*W1CW+j]
    return w.reshape(KO, 128, N_W1C, W1CW).transpose(2, 1, 0, 3)


def _tile_w2(w):
    # [I, H] -> [N_W2C, 128, KO, W2CW]
    return w.reshape(KO, 128, N_W2C, W2CW).transpose(2, 1, 0, 3)


def kernel(x, w1_shared, b1_shared, w2_shared, b2_shared,
           router_w, router_b, w1, b1, w2, b2):
    from concourse.bass_utils import run_bass_kernel_spmd

    wdt = WORK_DTYPE
    ndt = _np_wdt(wdt)

    x = np.asarray(x, np.float32)
    w1 = np.asarray(w1, np.float32)
    b1 = np.asarray(b1, np.float32)
    w2 = np.asarray(w2, np.float32)
    b2 = np.asarray(b2, np.float32)
    w1_shared = np.asarray(w1_shared, np.float32)
    b1_shared = np.asarray(b1_shared, np.float32)
    w2_shared = np.asarray(w2_shared, np.float32)
    b2_shared = np.asarray(b2_shared, np.float32)
    router_w = np.asarray(router_w, np.float32)
    router_b = np.asarray(router_b, np.float32)

    xf = x.reshape(T, HID)

    # ---------------- host routing ----------------
    logits = xf @ router_w + router_b
    m = logits.max(-1, keepdims=True)
    ex = np.exp(logits - m, dtype=np.float32)
    affin = ex / ex.sum(-1, keepdims=True, dtype=np.float32)
    order = np.argsort(-affin, axis=-1, kind="stable")[:, :TOP_K]   # [T, K]
    vals = np.take_along_axis(affin, order, axis=-1)                # [T, K]

    # group (token, gate) pairs by expert
    flat_e = order.ravel()
    flat_t = np.repeat(np.arange(T), TOP_K)
    flat_g = vals.ravel()
    sort = np.argsort(flat_e, kind="stable")
    se, st, sg = flat_e[sort], flat_t[sort], flat_g[sort]
    starts = np.searchsorted(se, np.arange(E + 1))
    tok_by_e = [st[starts[e]:starts[e + 1]] for e in range(E)]
    gate_by_e = [sg[starts[e]:starts[e + 1]] for e in range(E)]

    # slot table: 64 expert slots; slot s -> (expert, token idx, gates)
    slot_expert = [s if s < E else -1 for s in range(NCORES * 8)]
    slot_tok = [tok_by_e[s][:C] if s < E else np.empty(0, np.int64)
                for s in range(NCORES * 8)]
    slot_gate = [gate_by_e[s][:C] if s < E else np.empty(0, np.float32)
                 for s in range(NCORES * 8)]

    # overflow: worst-overflowing expert spills into slot 63; the rest go to
    # an exact host fallback (rare for any randomly-routed input).
    host_fallback = []  # (expert, tokens, gates)
    over = [e for e in range(E) if len(tok_by_e[e]) > C]
    if over:
        over.sort(key=lambda e: len(tok_by_e[e]), reverse=True)
        e0 = over[0]
        slot_expert[E] = e0
        slot_tok[E] = tok_by_e[e0][C:2 * C]
        slot_gate[E] = gate_by_e[e0][C:2 * C]
        if len(tok_by_e[e0]) > 2 * C:
            host_fallback.append((e0, tok_by_e[e0][2 * C:], gate_by_e[e0][2 * C:]))
        for e in over[1:]:
            host_fallback.append((e, tok_by_e[e][C:], gate_by_e[e][C:]))

    # ---------------- build per-core device inputs ----------------
    # x transposed + partition-tiled: xT_t[ko, p, t] = x[t, ko*128+p]
    xT_t = np.ascontiguousarray(xf.T).astype(ndt).reshape(KO, 128, T)

    w1t_sh = _tile_w1(w1_shared[0]).astype(ndt)
    w2t_sh = _tile_w2(w2_shared[0]).astype(ndt)
    b1t_sh = b1_shared[0].reshape(KO, 128).T

    in_maps = []
    for c in range(NCORES):
        xg = np.zeros((UNITS, 128, KO, C), ndt)
        gates = np.zeros((UNITS, 128, CM), np.float32)
        w1u = np.zeros((UNITS, N_W1C, 128, KO, W1CW), ndt)
        b1u = np.zeros((UNITS, 128, KO), np.float32)
        w2u = np.zeros((UNITS, N_W2C, 128, KO, W2CW), ndt)
        for u in range(8):
            s = c * 8 + u
            e = slot_expert[s]
            if e < 0 or len(slot_tok[s]) == 0:
                continue
            n = len(slot_tok[s])
            idx = np.zeros(C, np.int64)
            idx[:n] = slot_tok[s]
            xg[u] = xT_t[:, :, idx].swapaxes(0, 1)
            g = np.zeros(C, np.float32)
            g[:n] = slot_gate[s]
            gates[u] = g.reshape(CM, 128).T
            w1u[u] = _tile_w1(w1[e]).astype(ndt)
            b1u[u] = b1[e].reshape(KO, 128).T
            w2u[u] = _tile_w2(w2[e]).astype(ndt)
        # shared-expert unit
        xg[8] = xT_t[:, :, c * TSH:(c + 1) * TSH].swapaxes(0, 1)
        gates[8] = 1.0
        w1u[8] = w1t_sh
        b1u[8] = b1t_sh
        w2u[8] = w2t_sh
        in_maps.append({"xg": xg, "gates": gates, "w1": w1u, "b1": b1u, "w2": w2u})

    # ---------------- run on 8 cores ----------------
    nc = _get_nc(wdt)
    res = run_bass_kernel_spmd(nc, in_maps, core_ids=list(range(NCORES)))
    outs = [r["out"] for r in res.results]   # [UNITS, 128, CM, HID] each

    # ---------------- host unshard / scatter ----------------
    acc = np.zeros((T, HID), np.float32)     # shared + routed
    # shared expert (unit 8 on each core), gate 1, + b2_shared
    for c in range(NCORES):
        ys = outs[c][8].transpose(1, 0, 2).reshape(TSH, HID)
        acc[c * TSH:(c + 1) * TSH] = ys + b2_shared[0]
    # routed experts: add gated slot outputs + gate * b2[e]
    for s in range(NCORES * 8):
        e = slot_expert[s]
        n = len(slot_tok[s])
        if e < 0 or n == 0:
            continue
        ye = outs[s // 8][s % 8].transpose(1, 0, 2).reshape(C, HID)[:n]
        # token indices are unique within one slot, so fancy += is safe
        acc[slot_tok[s]] += ye + slot_gate[s][:, None] * b2[e][None, :]
    # exact host fallback for overflow beyond device capacity
    for e, toks, gs in host_fallback:
        h = _gelu_np(xf[toks] @ w1[e] + b1[e])
        acc[toks] += gs[:, None] * (h @ w2[e] + b2[e])

    return (acc + xf).reshape(B, S, HID).astype(np.float32)


# revision 14
# speedup vs baseline: 1.3495x; 1.0326x over previous
"""MoE (63 routed experts, top-7, 1 shared expert) Trainium2 Bass kernel.

Strategy (expert parallelism, per sharding hint):
  - Host: router matmul + softmax + top-k (tiny: 0.7 GFLOP vs 220 GFLOP of
    expert FFNs), token gather per expert.
  - Device (8 NeuronCores, SPMD): each core runs 9 "units" of identical
    shape: 8 routed-expert slots (64 slots globally = 63 experts + 1
    overflow slot) and 1 shared-expert slot over a 1/8 token slice.
    Each unit: h = gelu(XeT^T @ W1 + b1); y = gate * (h @ W2), with
    full-rate matmuls (float32r or bf16), GELU fused into the PSUM
    eviction on the scalar engine, gating fused into the PSUM eviction on
    the vector engine.  Weights are host-pretiled into chunk-contiguous
    layout so every DMA is a flat [128 x bytes] block.
  - Host: scatter-add gated expert outputs (+ gate*b2), add shared out,
    bias and residual.

Capacity C=512 tokens/slot (seed-0 max expert load is 520; the single
largest-overflow expert spills into the overflow slot; anything beyond that
falls back to an exact host-side FFN for the few excess tokens).
"""

import os

import numpy as np

B, S, HID = 2, 2048, 1280
E = 63
I = 1280
TOP_K = 7
NCORES = 8
UNITS = 9          # 8 expert slots + 1 shared-expert slot
C = 512            # token capacity per expert slot
CM = C // 128      # 4
KO = HID // 128    # 10 contraction chunks
T = B * S          # 4096
TSH = T // NCORES  # 512 shared-expert tokens per core

W1CW = 256          # w1 chunk width along I (2 lhsT column groups)
W2CW = 320          # w2 chunk width along H (psum free dim for mm2)
N_W1C = I // W1CW   # 5
N_W2C = HID // W2CW  # 4

# Per-unit-index token capacities. Experts are assigned to slots by load
# rank (rank r -> core r%8, unit r//8), so unit j only ever sees the j-th
# bucket of the descending load distribution; caps cover the bucket maxima
# of any near-uniform routing with margin. Uncovered overflow goes to the
# spare slot 63 and, beyond that, to an exact host fallback.
CAPS = [512, 512, 496, 480, 464, 448, 448, 432, C]   # unit 8 = shared

# "f32r": fp32 data, full-rate float32r matmuls (most accurate).
# "bf16": bf16 weights+activations, fp32 accumulate (halves DMA traffic).
# "fp16": like bf16 but 4x finer mantissa; all values here are well within
#         fp16 range, so this is strictly more accurate at the same speed.
WORK_DTYPE = os.environ.get("MOE_WDT", "fp16")

_cache = {}


def _build_nc(wdt):
    import concourse.mybir as mybir
    import concourse.tile as tile
    from concourse import bacc

    f32 = mybir.dt.float32
    GELU = mybir.ActivationFunctionType.Gelu
    if wdt == "f32r":
        mdt = mybir.dt.float32r
        ddt = f32    # dram dtype for weight/activation tensors
        bufs = dict(xu=2, h1=2, w1c=3, w2c=3, ou=2)
    else:
        mdt = mybir.dt.float16 if wdt == "fp16" else mybir.dt.bfloat16
        ddt = mdt
        bufs = dict(xu=3, h1=3, w1c=4, w2c=6, ou=6)

    nc = bacc.Bacc(None, target_bir_lowering=False)

    xg_d = nc.dram_tensor("xg", [UNITS, 128, KO, C], ddt, kind="ExternalInput")
    gates_d = nc.dram_tensor("gates", [UNITS, 128, CM], f32, kind="ExternalInput")
    w1_d = nc.dram_tensor("w1", [UNITS, N_W1C, 128, KO, W1CW], ddt,
                          kind="ExternalInput")
    b1_d = nc.dram_tensor("b1", [UNITS, 128, KO], f32, kind="ExternalInput")
    w2_d = nc.dram_tensor("w2", [UNITS, N_W2C, 128, KO, W2CW], ddt,
                          kind="ExternalInput")
    out_d = nc.dram_tensor("out", [UNITS, 128, CM, HID], f32, kind="ExternalOutput")

    def cast(ap):
        return ap.bitcast(mdt) if wdt == "f32r" else ap

    with tile.TileContext(nc) as tc:
        with tc.tile_pool(name="xg_p", bufs=bufs["xu"]) as xg_p, \
             tc.tile_pool(name="h1_p", bufs=bufs["h1"]) as h1_p, \
             tc.tile_pool(name="w1_p", bufs=bufs["w1c"]) as w1_p, \
             tc.tile_pool(name="w2_p", bufs=bufs["w2c"]) as w2_p, \
             tc.tile_pool(name="out_p", bufs=bufs["ou"]) as out_p, \
             tc.tile_pool(name="sm_p", bufs=3) as sm_p, \
             tc.tile_pool(name="ps1_p", bufs=3, space="PSUM") as ps1_p, \
             tc.tile_pool(name="ps2_p", bufs=4, space="PSUM") as ps2_p:

            for u in range(UNITS):
                CAP = CAPS[u]
                xu = xg_p.tile([128, KO, C], mdt, tag="xu")
                # split halves so the first matmuls can start sooner
                nc.sync.dma_start(xu[:, :KO // 2, :CAP],
                                  cast(xg_d[u, :, :KO // 2, :CAP]))
                nc.sync.dma_start(xu[:, KO // 2:, :CAP],
                                  cast(xg_d[u, :, KO // 2:, :CAP]))
                gu = sm_p.tile([128, CM], f32, tag="gu")
                nc.sync.dma_start(gu[:], gates_d[u])
                b1u = sm_p.tile([128, KO], f32, tag="b1u")
                nc.sync.dma_start(b1u[:], b1_d[u])

                h1 = h1_p.tile([128, KO, C], mdt, tag="h1")

                # ---- mm1: h1[i, c] = gelu(sum_h W1[h,i] * X^T[h,c] + b1[i])
                for ic in range(N_W1C):
                    w1c = w1_p.tile([128, KO, W1CW], mdt, tag="w1c")
                    nc.sync.dma_start(w1c[:], cast(w1_d[u, ic]))
                    for s in range(W1CW // 128):
                        i_out = ic * (W1CW // 128) + s
                        ps = ps1_p.tile([128, C], f32, tag="ps1")
                        for ko in range(KO):
                            nc.tensor.matmul(
                                ps[:, :CAP],
                                w1c[:, ko, s * 128:(s + 1) * 128],
                                xu[:, ko, :CAP],
                                start=(ko == 0),
                                stop=(ko == KO - 1),
                            )
                        nc.scalar.activation(
                            h1[:, i_out, :CAP], ps[:, :CAP], GELU,
                            bias=b1u[:, i_out:i_out + 1])

                # ---- mm2: y[c, h] = gate[c] * sum_i h1[i, c] * W2[i, h]
                w2cs = []
                for hc in range(N_W2C):
                    w2c = w2_p.tile([128, KO, W2CW], mdt, tag="w2c")
                    nc.sync.dma_start(w2c[:], cast(w2_d[u, hc]))
                    w2cs.append(w2c)
                for ci in range((CAP + 127) // 128):
                    mw = min(128, CAP - ci * 128)
                    ou = out_p.tile([128, HID], f32, tag="ou")
                    for hc in range(N_W2C):
                        ps2 = ps2_p.tile([128, W2CW], f32, tag="ps2")
                        for ko in range(KO):
                            nc.tensor.matmul(
                                ps2[:mw, :],
                                h1[:, ko, ci * 128:ci * 128 + mw],
                                w2cs[hc][:, ko, :],
                                start=(ko == 0),
                                stop=(ko == KO - 1),
                            )
                        nc.vector.tensor_scalar_mul(
                            ou[:mw, hc * W2CW:(hc + 1) * W2CW], ps2[:mw, :],
                            gu[:mw, ci:ci + 1])
                    nc.sync.dma_start(out_d[u, :mw, ci, :], ou[:mw, :])

    nc.compile()
    return nc


def _get_nc(wdt):
    if wdt not in _cache:
        _cache[wdt] = _build_nc(wdt)
    return _cache[wdt]


def _np_wdt(wdt):
    if wdt == "bf16":
        import ml_dtypes
        return np.dtype(ml_dtypes.bfloat16)
    if wdt == "fp16":
        return np.dtype(np.float16)
    return np.dtype(np.float32)


def _gelu_np(v):
    from scipy.special import erf
    v = v.astype(np.float32)
    return (0.5 * v * (1.0 + erf(v / np.sqrt(2.0)))).astype(np.float32)


def _tile_w1(w):
    # [H, I] -> [N_W1C, 128, KO, W1CW] with w1t[ic, p, ko, j] = w[ko*128+p, ic*W1CW+j]
    return w.reshape(KO, 128, N_W1C, W1CW).transpose(2, 1, 0, 3)


def _tile_w2(w):
    # [I, H] -> [N_W2C, 128, KO, W2CW]
    return w.reshape(KO, 128, N_W2C, W2CW).transpose(2, 1, 0, 3)


def kernel(x, w1_shared, b1_shared, w2_shared, b2_shared,
           router_w, router_b, w1, b1, w2, b2):
    from concourse.bass_utils import run_bass_kernel_spmd

    wdt = WORK_DTYPE
    ndt = _np_wdt(wdt)

    x = np.asarray(x, np.float32)
    w1 = np.asarray(w1, np.float32)
    b1 = np.asarray(b1, np.float32)
    w2 = np.asarray(w2, np.float32)
    b2 = np.asarray(b2, np.float32)
    w1_shared = np.asarray(w1_shared, np.float32)
    b1_shared = np.asarray(b1_shared, np.float32)
    w2_shared = np.asarray(w2_shared, np.float32)
    b2_shared = np.asarray(b2_shared, np.float32)
    router_w = np.asarray(router_w, np.float32)
    router_b = np.asarray(router_b, np.float32)

    xf = x.reshape(T, HID)

    # ---------------- host routing ----------------
    logits = xf @ router_w + router_b
    m = logits.max(-1, keepdims=True)
    ex = np.exp(logits - m, dtype=np.float32)
    affin = ex / ex.sum(-1, keepdims=True, dtype=np.float32)
    order = np.argsort(-affin, axis=-1, kind="stable")[:, :TOP_K]   # [T, K]
    vals = np.take_along_axis(affin, order, axis=-1)                # [T, K]

    # group (token, gate) pairs by expert
    flat_e = order.ravel()
    flat_t = np.repeat(np.arange(T), TOP_K)
    flat_g = vals.ravel()
    sort = np.argsort(flat_e, kind="stable")
    se, st, sg = flat_e[sort], flat_t[sort], flat_g[sort]
    starts = np.searchsorted(se, np.arange(E + 1))
    tok_by_e = [st[starts[e]:starts[e + 1]] for e in range(E)]
    gate_by_e = [sg[starts[e]:starts[e + 1]] for e in range(E)]

    # slot table: 64 expert slots; slot s = core*8 + unit.  Experts are
    # assigned by descending load rank: rank r -> core r%8, unit r//8, so
    # every core gets one expert from each load bucket and unit j's static
    # capacity CAPS[j] covers its bucket maximum.
    NSLOT = NCORES * 8
    slot_expert = [-1] * NSLOT
    slot_tok = [np.empty(0, np.int64)] * NSLOT
    slot_gate = [np.empty(0, np.float32)] * NSLOT
    ranked = sorted(range(E), key=lambda e: -len(tok_by_e[e]))
    overflow = []   # (expert, tokens, gates) beyond the primary slot cap
    for r, e in enumerate(ranked):
        s = (r % NCORES) * 8 + (r // NCORES)
        cap = CAPS[r // NCORES]
        slot_expert[s] = e
        slot_tok[s] = tok_by_e[e][:cap]
        slot_gate[s] = gate_by_e[e][:cap]
        if len(tok_by_e[e]) > cap:
            overflow.append((e, tok_by_e[e][cap:], gate_by_e[e][cap:]))
    # worst overflow spills into the spare slot 63 (unit 7, cap CAPS[7]);
    # anything further goes to an exact host fallback (rare).
    host_fallback = []
    if overflow:
        overflow.sort(key=lambda t: -len(t[1]))
        e0, t0, g0 = overflow[0]
        cap63 = CAPS[7]
        slot_expert[63] = e0
        slot_tok[63] = t0[:cap63]
        slot_gate[63] = g0[:cap63]
        if len(t0) > cap63:
            host_fallback.append((e0, t0[cap63:], g0[cap63:]))
        for e, t, g in overflow[1:]:
            host_fallback.append((e, t, g))

    # ---------------- build per-core device inputs ----------------
    # x transposed + partition-tiled: xT_t[ko, p, t] = x[t, ko*128+p]
    xT_t = np.ascontiguousarray(xf.T).astype(ndt).reshape(KO, 128, T)

    w1t_sh = _tile_w1(w1_shared[0]).astype(ndt)
    w2t_sh = _tile_w2(w2_shared[0]).astype(ndt)
    b1t_sh = b1_shared[0].reshape(KO, 128).T

    in_maps = []
    for c in range(NCORES):
        xg = np.zeros((UNITS, 128, KO, C), ndt)
        gates = np.zeros((UNITS, 128, CM), np.float32)
        w1u = np.zeros((UNITS, N_W1C, 128, KO, W1CW), ndt)
        b1u = np.zeros((UNITS, 128, KO), np.float32)
        w2u = np.zeros((UNITS, N_W2C, 128, KO, W2CW), ndt)
        for u in range(8):
            s = c * 8 + u
            e = slot_expert[s]
            if e < 0 or len(slot_tok[s]) == 0:
                continue
            n = len(slot_tok[s])
            idx = np.zeros(C, np.int64)
            idx[:n] = slot_tok[s]
            xg[u] = xT_t[:, :, idx].swapaxes(0, 1)
            g = np.zeros(C, np.float32)
            g[:n] = slot_gate[s]
            gates[u] = g.reshape(CM, 128).T
            w1u[u] = _tile_w1(w1[e]).astype(ndt)
            b1u[u] = b1[e].reshape(KO, 128).T
            w2u[u] = _tile_w2(w2[e]).astype(ndt)
        # shared-expert unit
        xg[8] = xT_t[:, :, c * TSH:(c + 1) * TSH].swapaxes(0, 1)
        gates[8] = 1.0
        w1u[8] = w1t_sh
        b1u[8] = b1t_sh
        w2u[8] = w2t_sh
        in_maps.append({"xg": xg, "gates": gates, "w1": w1u, "b1": b1u, "w2": w2u})

    # ---------------- run on 8 cores ----------------
    nc = _get_nc(wdt)
    res = run_bass_kernel_spmd(nc, in_maps, core_ids=list(range(NCORES)))
    outs = [r["out"] for r in res.results]   # [UNITS, 128, CM, HID] each

    # ---------------- host unshard / scatter ----------------
    acc = np.zeros((T, HID), np.float32)     # shared + routed
    # shared expert (unit 8 on each core), gate 1, + b2_shared
    for c in range(NCORES):
        ys = outs[c][8].transpose(1, 0, 2).reshape(TSH, HID)
        acc[c * TSH:(c + 1) * TSH] = ys + b2_shared[0]
    # routed experts: add gated slot outputs + gate * b2[e]
    for s in range(NCORES * 8):
        e = slot_expert[s]
        n = len(slot_tok[s])
        if e < 0 or n == 0:
            continue
        ye = outs[s // 8][s % 8].transpose(1, 0, 2).reshape(C, HID)[:n]
        # token indices are unique within one slot, so fancy += is safe
        acc[slot_tok[s]] += ye + slot_gate[s][:, None] * b2[e][None, :]
    # exact host fallback for overflow beyond device capacity
    for e, toks, gs in host_fallback:
        h = _gelu_np(xf[toks] @ w1[e] + b1[e])
        acc[toks] += gs[:, None] * (h @ w2[e] + b2[e])

    return (acc + xf).reshape(B, S, HID).astype(np.float32)
